# revision 29
# baseline (speedup 1.0000x reference)
r"""GCN block (gather -> normalize -> scatter-add -> linear -> relu) on 8 trn2 cores.

Math: out = relu( \hat{A} (X W) + b ) with \hat{A} = D^-1/2 (A + I) D^-1/2,
degree over destination of (edges + self loops).

v3 "materialized identity-stream" design:
  The norm factorizes: norm(e) = dinv[src] * dinv[dst]. Fold dinv[src] into a
  host-prescaled table x' = dinv[:,None] * x (fp16), and dinv[dst] into a
  per-window constant diagonal rhs. Self loops become ordinary messages
  (src == dst, rank 0 of each dst).

  Host routing (per core, 12500 dst nodes = 98 windows of 128):
   - message m = k-th in-message of dst d (self loop first). If k < T (=14),
     m rides IDENTITY chunk k of d's window at slot = d%128: the scatter
     matmul rhs is the CONSTANT diag(dinv of the window's dsts), so no
     per-chunk DVE build and no per-message index on the device.
   - k >= T messages go to per-window OVERFLOW chunks (dense, any slot) with
     a classic one-hot rhs (iota==dst_off)*dinv[dst] built by tensor_scalar.
   - The whole message stream (identity + overflow chunk slots, zero rows for
     padding) is MATERIALIZED on the host, transposed to stream_t
     [128 slots, C*128 ch] fp16, so the device "gather" is a plain sequential
     HWDGE dma_start per PSUM group (~2.4MB each, full HBM bandwidth; no
     SWDGE descriptor-issue bottleneck, which limited the previous design to
     ~1.25ms at ~1.42us per 128-descriptor indirect-DMA call).

  Device per PSUM group (4 windows = one 512-col PSUM bank):
   - 1 dma_start pulls the group's chunk slab into SBUF (first SPLIT_G
     groups fetch per-window: SDMA round-robins across ALL queued
     transfers at packet granularity, so small early slabs complete fast
     and cut the PE startup stall from ~25us to ~13us; slab DMAs alternate
     between the two HWDGE engines SP/ACT)
   - per window: diag rhs built just-in-time (tensor_scalar, DVE — NOT
     gpsimd: Pool tensor_scalar is a ~2.2us software op and serialized the
     whole kernel at 1.18ms), T identity matmuls + K_w overflow matmuls
     accumulate ps1[ch, dst] (PE, fp16, 128 cycles each)
   - epilogue: ps1 -> fp16 agg (ACT copy), ps2 = W^T-form matmul, relu+bias
     on ACT, DMA out [ch, dst] in fp16; host transposes/casts back.

Program shape depends only on the cross-core per-window overflow chunk
counts (k-table); identity chunk count T is fixed.

kernel() re-verifies each device run against a 512-row host recompute and
retries (rare first-execution DMA/engine race produced one corrupted run
at rel~8e-3 during development).

Measured on 8 trn2 cores: 187-206us HW exec depending on device
power-throttle state (6.1-6.7x over the 1.25ms SWDGE-gather baseline),
rel L2 err ~4.6e-4. Bound by the stream read: 57.4MB/core at ~375GB/s =
153us pure-DMA floor (measured), plus 3.2MB output write and ~20us of
power throttling (throttle counters in trace). Tensor ~120us and DVE
~122us busy both fit under the DMA shadow.
"""

import sys
from contextlib import ExitStack
from dataclasses import dataclass

import numpy as np

if "/opt/trn_rl_repo" not in sys.path:
    sys.path.insert(0, "/opt/trn_rl_repo")

import concourse.bass as bass
import concourse.bacc as bacc
import concourse.mybir as mybir
import concourse.tile as tile
from concourse.bass_utils import run_bass_kernel_spmd


def _ensure_axon_hooks_stub():
    """The image's antenv package lacks axon_hooks; bass_utils imports it on
    the trace path (e.g. when BASS_TRACE is set). Provide a stub returning
    None so tracing degrades gracefully instead of raising ImportError."""
    import types

    name = "antenv.axon_hooks"
    if name in sys.modules:
        return
    try:
        __import__(name)
        return
    except ImportError:
        pass
    mod = types.ModuleType(name)
    mod._hook = None
    mod.set_axon_ntff_profile_hook = lambda h: setattr(mod, "_hook", h)
    mod.get_axon_ntff_profile_hook = lambda: mod._hook
    sys.modules[name] = mod
    try:
        import antenv

        antenv.axon_hooks = mod
    except ImportError:
        pass


_ensure_axon_hooks_stub()

P = 128
T_ID = 16  # identity chunks per window (covers the first T_ID msgs of each dst)
GRP = 4  # windows per PSUM group
OBATCH = 4  # PSUM groups per output-write DMA


@dataclass(frozen=True)
class Cfg:
    n_nodes: int = 100000
    in_ch: int = 128
    out_ch: int = 128
    m: int = 8  # cores

    @property
    def np_per(self) -> int:
        return self.n_nodes // self.m

    @property
    def n_win(self) -> int:
        return (self.np_per + P - 1) // P


FULL = Cfg()


def route_edges(edge_index: np.ndarray, cfg: Cfg = FULL):
    """Host-side routing (indices only). Returns (k_ovf, per_core):
    k_ovf[w] = overflow chunks for window w (max over cores, len n_win);
    per_core[p] = dict of index arrays for make_in_maps:
      id_col/id_slot/id_src  — stream position of each identity message
      ov_col/ov_slot/ov_src/ov_off/ov_dinv — same for overflow messages
      (cols are *local* chunk ids before k-table padding: filled in later)
      plus dinv (full-table) for the caller."""
    n = cfg.n_nodes
    nw = cfg.n_win
    src = np.asarray(edge_index[0], dtype=np.int64)
    dst = np.asarray(edge_index[1], dtype=np.int64)

    deg = (np.bincount(dst, minlength=n) + 1).astype(np.float32)
    dinv = (1.0 / np.sqrt(deg, dtype=np.float32)).astype(np.float32)

    # messages = self loops first (rank 0 within each dst), then edges
    loop = np.arange(n, dtype=np.int64)
    msrc = np.concatenate([loop, src])
    mdst = np.concatenate([loop, dst])
    order = np.argsort(mdst, kind="stable")
    s_dst = mdst[order]
    s_src = msrc[order]
    # rank of each message within its dst (loops got rank 0)
    starts = np.searchsorted(s_dst, np.arange(n))
    rank = np.arange(len(s_dst), dtype=np.int64) - starts[s_dst]

    per_core = []
    k_real = np.zeros((cfg.m, nw), np.int64)
    for p in range(cfg.m):
        base = p * cfg.np_per
        lo = np.searchsorted(s_dst, base)
        hi = np.searchsorted(s_dst, base + cfg.np_per)
        d_loc = s_dst[lo:hi] - base
        c_src = s_src[lo:hi]
        c_rank = rank[lo:hi]
        w = d_loc >> 7
        slot = d_loc & 127

        idm = c_rank < T_ID
        id_w = w[idm]
        id_chunk = c_rank[idm]  # chunk-in-window (0..T_ID-1)
        id_slot = slot[idm]
        id_src = c_src[idm]

        ovm = ~idm
        ov_w = w[ovm]  # sorted ascending (messages sorted by dst)
        ov_src = c_src[ovm]
        ov_dst = d_loc[ovm]
        wstart = np.searchsorted(ov_w, np.arange(nw))
        pos = np.arange(len(ov_w), dtype=np.int64) - wstart[ov_w]
        ov_chunk = pos >> 7
        ov_slot = pos & 127
        k_real[p] = np.ceil(np.bincount(ov_w, minlength=nw) / P).astype(np.int64)

        per_core.append(
            dict(
                id_w=id_w,
                id_chunk=id_chunk,
                id_slot=id_slot,
                id_src=id_src,
                ov_w=ov_w,
                ov_chunk=ov_chunk,
                ov_slot=ov_slot,
                ov_src=ov_src,
                ov_off=(ov_dst & 127),
                ov_dinv=dinv[ov_dst + base],
            )
        )

    k_ovf = k_real.max(axis=0)  # [n_win]
    # s_dst/s_src kept for the post-run sample check in kernel()
    per_core.append(dict(s_dst=s_dst, s_src=s_src))
    return k_ovf, per_core, dinv


def build_program(k_ovf, cfg: Cfg = FULL, sdt=mybir.dt.float16):
    """Build + compile the SPMD bass program (identical on all cores)."""
    nw = cfg.n_win
    k_ovf = np.asarray(k_ovf, dtype=np.int64)
    c_tot = int(nw * T_ID + k_ovf.sum())
    c_ovf = int(k_ovf.sum())
    n_grp = (nw + GRP - 1) // GRP

    nc = bacc.Bacc(
        "TRN2",
        target_bir_lowering=False,
        debug=False,
        enable_asserts=False,
        num_devices=cfg.m,
    )
    f32 = mybir.dt.float32
    stream_t = nc.dram_tensor("stream_t", [P, c_tot * P], sdt, kind="ExternalInput").ap()
    do_in = nc.dram_tensor("do_ovf", [P, max(c_ovf, 1)], f32, kind="ExternalInput").ap()
    nv_in = nc.dram_tensor("nv_ovf", [P, max(c_ovf, 1)], f32, kind="ExternalInput").ap()
    d2_in = nc.dram_tensor("d2", [P, nw], f32, kind="ExternalInput").ap()
    io_in = nc.dram_tensor("iota", [P, P], sdt, kind="ExternalInput").ap()
    ioc_in = nc.dram_tensor("iotac", [P, 1], f32, kind="ExternalInput").ap()
    w_in = nc.dram_tensor("w", [cfg.in_ch, cfg.out_ch], sdt, kind="ExternalInput").ap()
    b_in = nc.dram_tensor("b", [P, 1], f32, kind="ExternalInput").ap()
    out_t = nc.dram_tensor("out_t", [P, nw * P], sdt, kind="ExternalOutput").ap()

    with tile.TileContext(nc) as tc:
        with ExitStack() as ctx:
            cpool = ctx.enter_context(tc.tile_pool(name="const", bufs=1))
            gpool = ctx.enter_context(tc.tile_pool(name="gather", bufs=6))
            ohpool = ctx.enter_context(tc.tile_pool(name="oh", bufs=24))
            aggpool = ctx.enter_context(tc.tile_pool(name="agg", bufs=4))
            outpool = ctx.enter_context(tc.tile_pool(name="outp", bufs=4))
            pp1 = ctx.enter_context(tc.tile_pool(name="ps1", bufs=4, space="PSUM"))
            pp2 = ctx.enter_context(tc.tile_pool(name="ps2", bufs=2, space="PSUM"))

            do = cpool.tile([P, max(c_ovf, 1)], f32)
            nv = cpool.tile([P, max(c_ovf, 1)], f32)
            d2 = cpool.tile([P, nw], f32)
            io = cpool.tile([P, P], sdt)
            ioc = cpool.tile([P, 1], f32)
            wt = cpool.tile([P, cfg.out_ch], sdt)
            bb = cpool.tile([P, 1], f32)
            nc.sync.dma_start(out=do[:], in_=do_in[:])
            nc.sync.dma_start(out=nv[:], in_=nv_in[:])
            nc.sync.dma_start(out=d2[:], in_=d2_in[:])
            nc.sync.dma_start(out=io[:], in_=io_in[:])
            nc.sync.dma_start(out=ioc[:], in_=ioc_in[:])
            nc.sync.dma_start(out=wt[:], in_=w_in[:])
            nc.sync.dma_start(out=bb[:], in_=b_in[:])

            # one diag tile per window, built just-in-time (inline, so DVE
            # program order interleaves diag and overflow-oh builds); bufs=nw
            # so tiles are never recycled (no WAR serialization)
            dgpool = ctx.enter_context(tc.tile_pool(name="dg", bufs=nw))

            col = 0  # stream chunk column
            colk = 0  # overflow table column
            ndma = 0  # alternation counter for the two HWDGE engines
            # SDMA round-robins across ALL queued transfers at packet
            # granularity, so a big first slab behind a deep queue completes
            # late and stalls PE ~25us. Fetch the first SPLIT_G groups
            # per-window (small, early-completing slabs); group slabs after.
            SPLIT_G = 2
            for gi in range(n_grp):
                wls = list(range(gi * GRP, min((gi + 1) * GRP, nw)))
                gw = len(wls) * P
                # (tile, chunk base within tile) per window of this group
                tiles = []
                if gi < SPLIT_G:
                    for w in wls:
                        kw_tot = T_ID + int(k_ovf[w])
                        wt_t = gpool.tile([P, kw_tot * P], sdt)
                        deng = nc.sync if ndma % 2 == 0 else nc.scalar
                        ndma += 1
                        deng.dma_start(
                            out=wt_t[:],
                            in_=stream_t[:, col * P : (col + kw_tot) * P],
                        )
                        tiles.append((wt_t, 0))
                        col += kw_tot
                else:
                    kg = sum(T_ID + int(k_ovf[w]) for w in wls)
                    gt = gpool.tile([P, kg * P], sdt)
                    deng = nc.sync if ndma % 2 == 0 else nc.scalar
                    ndma += 1
                    deng.dma_start(
                        out=gt[:], in_=stream_t[:, col * P : (col + kg) * P]
                    )
                    base = 0
                    for w in wls:
                        tiles.append((gt, base))
                        base += T_ID + int(k_ovf[w])
                    col += kg
                ps1 = pp1.tile([P, gw], mybir.dt.float32, space="PSUM")
                for wl, w in enumerate(wls):
                    kw = int(k_ovf[w])
                    wtile, cc = tiles[wl]
                    dgt = dgpool.tile([P, P], sdt)
                    nc.vector.tensor_scalar(
                        out=dgt[:],
                        in0=io[:],
                        scalar1=ioc[:],
                        scalar2=d2[:, w : w + 1],
                        op0=mybir.AluOpType.is_equal,
                        op1=mybir.AluOpType.mult,
                    )
                    for k in range(T_ID):
                        nc.tensor.matmul(
                            ps1[:, wl * P : (wl + 1) * P],
                            lhsT=wtile[:, cc * P : (cc + 1) * P],
                            rhs=dgt[:],
                            start=(k == 0),
                            stop=(k == T_ID - 1 and kw == 0),
                        )
                        cc += 1
                    for c in range(kw):
                        oh = ohpool.tile([P, P], sdt)
                        nc.vector.tensor_scalar(
                            out=oh[:],
                            in0=io[:],
                            scalar1=do[:, colk + c : colk + c + 1],
                            scalar2=nv[:, colk + c : colk + c + 1],
                            op0=mybir.AluOpType.is_equal,
                            op1=mybir.AluOpType.mult,
                        )
                        nc.tensor.matmul(
                            ps1[:, wl * P : (wl + 1) * P],
                            lhsT=wtile[:, cc * P : (cc + 1) * P],
                            rhs=oh[:],
                            start=False,
                            stop=(c == kw - 1),
                        )
                        cc += 1
                    colk += kw
                agg = aggpool.tile([P, gw], sdt)
                nc.scalar.copy(out=agg[:], in_=ps1[:])
                ps2 = pp2.tile([P, gw], mybir.dt.float32, space="PSUM")
                nc.tensor.matmul(ps2[:], lhsT=wt[:], rhs=agg[:], start=True, stop=True)
                # batch output writes: 4 groups (16 windows, 512KB) per DMA —
                # 128KB out-DMAs only reach ~180GB/s on the SDMA engines
                if gi % OBATCH == 0:
                    ob_w0 = wls[0]
                    ob_wn = min(nw, ob_w0 + OBATCH * GRP)
                    obuf = outpool.tile([P, (ob_wn - ob_w0) * P], sdt)
                nc.scalar.activation(
                    out=obuf[:, (wls[0] - ob_w0) * P : (wls[0] - ob_w0) * P + gw],
                    in_=ps2[:],
                    func=mybir.ActivationFunctionType.Relu,
                    bias=bb[:],
                    scale=1.0,
                )
                if gi % OBATCH == OBATCH - 1 or gi == n_grp - 1:
                    (nc.scalar if gi % 2 == 0 else nc.sync).dma_start(
                        out=out_t[:, ob_w0 * P : ob_wn * P], in_=obuf[:]
                    )

    nc.compile()
    return nc


def make_in_maps(x, W, b, k_ovf, per_core, dinv, cfg: Cfg = FULL, np_sdt=np.float16):
    nw = cfg.n_win
    k_ovf = np.asarray(k_ovf, dtype=np.int64)
    c_tot = int(nw * T_ID + k_ovf.sum())
    c_ovf = int(k_ovf.sum())
    # column base of window w's identity block in the stream; overflow block
    # follows immediately. Also the overflow-table column base per window.
    cumk = np.zeros(nw + 1, np.int64)
    np.cumsum(k_ovf, out=cumk[1:])
    col_base = T_ID * np.arange(nw, dtype=np.int64) + cumk[:-1]
    ovf_base = cumk[:-1]

    x32 = np.asarray(x, dtype=np.float32)
    x2 = (x32 * dinv[:, None]).astype(np_sdt)  # dinv[src]-prescaled table

    iota = np.broadcast_to(
        np.arange(P, dtype=np.float32), (P, P)
    ).astype(np_sdt).copy()
    iotac = np.arange(P, dtype=np.float32).reshape(P, 1).copy()
    w_np = np.ascontiguousarray(np.asarray(W, dtype=np.float32)).astype(np_sdt)
    b_np = np.asarray(b, dtype=np.float32).reshape(P, 1).copy()

    in_maps = []
    for p in range(cfg.m):
        r = per_core[p]
        base = p * cfg.np_per
        stream = np.zeros((c_tot, P, cfg.in_ch), np_sdt)
        icol = col_base[r["id_w"]] + r["id_chunk"]
        stream[icol, r["id_slot"]] = x2[r["id_src"]]
        ocol = col_base[r["ov_w"]] + T_ID + r["ov_chunk"]
        stream[ocol, r["ov_slot"]] = x2[r["ov_src"]]
        stream_t = np.ascontiguousarray(
            stream.transpose(1, 0, 2).reshape(P, c_tot * cfg.in_ch)
        )

        do_np = np.zeros((P, max(c_ovf, 1)), np.float32)
        nv_np = np.zeros((P, max(c_ovf, 1)), np.float32)
        okol = ovf_base[r["ov_w"]] + r["ov_chunk"]
        do_np[r["ov_slot"], okol] = r["ov_off"].astype(np.float32)
        nv_np[r["ov_slot"], okol] = r["ov_dinv"]

        d2_np = np.zeros((P, nw), np.float32)
        nn = cfg.np_per
        loc = np.arange(nn, dtype=np.int64)
        d2_np[loc & 127, loc >> 7] = dinv[base + loc]

        in_maps.append(
            dict(
                stream_t=stream_t,
                do_ovf=do_np,
                nv_ovf=nv_np,
                d2=d2_np,
                iota=iota,
                iotac=iotac,
                w=w_np,
                b=b_np,
            )
        )
    return in_maps


_PROG_CACHE = {}


def _sample_check(out, x, W, b, dinv, s_dst, s_src, n_samples=512, seed=7):
    """Host-recompute a random sample of output rows; returns True if the
    device output matches (guards against rare first-run DMA/engine races)."""
    n = out.shape[0]
    rng = np.random.default_rng(seed)
    samp = rng.choice(n, size=n_samples, replace=False)
    x32 = np.asarray(x, dtype=np.float32)
    w32 = np.asarray(W, dtype=np.float32)
    b32 = np.asarray(b, dtype=np.float32)
    starts = np.searchsorted(s_dst, samp)
    ends = np.searchsorted(s_dst, samp + 1)
    for d, lo, hi in zip(samp, starts, ends):
        srcs = s_src[lo:hi]
        agg = (x32[srcs] * dinv[srcs][:, None]).sum(axis=0) * dinv[d]
        exp = np.maximum(agg @ w32 + b32, 0.0)
        scale = max(float(np.linalg.norm(exp)), 1e-3)
        if float(np.linalg.norm(out[d] - exp)) > 0.02 * scale:
            return False
    return True


def kernel(x, edge_index, W, b):
    cfg = FULL
    k_ovf, per_core, dinv = route_edges(edge_index, cfg)
    aux = per_core[cfg.m]  # s_dst/s_src appended by route_edges
    key = (tuple(int(v) for v in k_ovf), cfg)
    if key not in _PROG_CACHE:
        _PROG_CACHE[key] = build_program(k_ovf, cfg)
    nc = _PROG_CACHE[key]
    in_maps = make_in_maps(x, W, b, k_ovf, per_core, dinv, cfg)
    out = np.empty((cfg.n_nodes, cfg.out_ch), np.float32)
    for attempt in range(3):
        res = run_bass_kernel_spmd(nc, in_maps, core_ids=list(range(cfg.m)))
        for p in range(cfg.m):
            out[p * cfg.np_per : (p + 1) * cfg.np_per] = (
                res.results[p]["out_t"][:, : cfg.np_per].T.astype(np.float32)
            )
        if _sample_check(out, x, W, b, dinv, aux["s_dst"], aux["s_src"]):
            break
        print(f"kernel: sample check failed (attempt {attempt}), re-running", flush=True)
    return out


# revision 32
# speedup vs baseline: 1.0304x; 1.0304x over previous
r"""GCN block (gather -> normalize -> scatter-add -> linear -> relu) on 8 trn2 cores.

Math: out = relu( \hat{A} (X W) + b ) with \hat{A} = D^-1/2 (A + I) D^-1/2,
degree over destination of (edges + self loops).

v3 "materialized identity-stream" design:
  The norm factorizes: norm(e) = dinv[src] * dinv[dst]. Fold dinv[src] into a
  host-prescaled table x' = dinv[:,None] * x (fp16), and dinv[dst] into a
  per-window constant diagonal rhs. Self loops become ordinary messages
  (src == dst, rank 0 of each dst).

  Host routing (per core, 12500 dst nodes = 98 windows of 128):
   - message m = k-th in-message of dst d (self loop first). If k < T (=14),
     m rides IDENTITY chunk k of d's window at slot = d%128: the scatter
     matmul rhs is the CONSTANT diag(dinv of the window's dsts), so no
     per-chunk DVE build and no per-message index on the device.
   - k >= T messages go to per-window OVERFLOW chunks (dense, any slot) with
     a classic one-hot rhs (iota==dst_off)*dinv[dst] built by tensor_scalar.
   - The whole message stream (identity + overflow chunk slots, zero rows for
     padding) is MATERIALIZED on the host, transposed to stream_t
     [128 slots, C*128 ch] fp16, so the device "gather" is a plain sequential
     HWDGE dma_start per PSUM group (~2.4MB each, full HBM bandwidth; no
     SWDGE descriptor-issue bottleneck, which limited the previous design to
     ~1.25ms at ~1.42us per 128-descriptor indirect-DMA call).

  Device per PSUM group (4 windows = one 512-col PSUM bank):
   - 1 dma_start pulls the group's chunk slab into SBUF (first SPLIT_G
     groups fetch per-window: SDMA round-robins across ALL queued
     transfers at packet granularity, so small early slabs complete fast
     and cut the PE startup stall from ~25us to ~13us; slab DMAs alternate
     between the two HWDGE engines SP/ACT)
   - per window: diag rhs built just-in-time (tensor_scalar, DVE — NOT
     gpsimd: Pool tensor_scalar is a ~2.2us software op and serialized the
     whole kernel at 1.18ms), T identity matmuls + K_w overflow matmuls
     accumulate ps1[ch, dst] (PE, fp16, 128 cycles each)
   - epilogue: ps1 -> fp16 agg (ACT copy), ps2 = W^T-form matmul, relu+bias
     on ACT, DMA out [ch, dst] in fp16; host transposes/casts back.

Program shape depends only on the cross-core per-window overflow chunk
counts (k-table); identity chunk count T is fixed.

kernel() re-verifies each device run against a 512-row host recompute and
retries (rare first-execution DMA/engine race produced one corrupted run
at rel~8e-3 during development).

Measured on 8 trn2 cores: 187-206us HW exec depending on device
power-throttle state (6.1-6.7x over the 1.25ms SWDGE-gather baseline),
rel L2 err ~4.6e-4. Bound by the stream read: 57.4MB/core at ~375GB/s =
153us pure-DMA floor (measured), plus 3.2MB output write and ~20us of
power throttling (throttle counters in trace). Tensor ~120us and DVE
~122us busy both fit under the DMA shadow.

Interleaved A/B sweeps (same device window) settled the knobs: OBATCH=4
(batched 512KB output writes) +10us, output DMAs on SWDGE/Pool +10-20us,
GRP=2 (finer PSUM groups) +10-20us, T_ID=16 and GBUFS 4 vs 6 within
noise. Round-to-round variance is ~+/-15us and moves all variants
together — device-level throttling, not scheduling.
"""

import sys
from contextlib import ExitStack
from dataclasses import dataclass

import numpy as np

if "/opt/trn_rl_repo" not in sys.path:
    sys.path.insert(0, "/opt/trn_rl_repo")

import concourse.bass as bass
import concourse.bacc as bacc
import concourse.mybir as mybir
import concourse.tile as tile
from concourse.bass_utils import run_bass_kernel_spmd


def _ensure_axon_hooks_stub():
    """The image's antenv package lacks axon_hooks; bass_utils imports it on
    the trace path (e.g. when BASS_TRACE is set). Provide a stub returning
    None so tracing degrades gracefully instead of raising ImportError."""
    import types

    name = "antenv.axon_hooks"
    if name in sys.modules:
        return
    try:
        __import__(name)
        return
    except ImportError:
        pass
    mod = types.ModuleType(name)
    mod._hook = None
    mod.set_axon_ntff_profile_hook = lambda h: setattr(mod, "_hook", h)
    mod.get_axon_ntff_profile_hook = lambda: mod._hook
    sys.modules[name] = mod
    try:
        import antenv

        antenv.axon_hooks = mod
    except ImportError:
        pass


_ensure_axon_hooks_stub()

P = 128
T_ID = 14  # identity chunks per window (covers the first T_ID msgs of each dst)
GRP = 4  # windows per PSUM group
OBATCH = 1  # PSUM groups per output-write DMA
OUT_ON_POOL = False  # issue output DMAs via SWDGE (idle Pool) instead of HWDGE
GBUFS = 6  # gather (stream slab) pool depth


@dataclass(frozen=True)
class Cfg:
    n_nodes: int = 100000
    in_ch: int = 128
    out_ch: int = 128
    m: int = 8  # cores

    @property
    def np_per(self) -> int:
        return self.n_nodes // self.m

    @property
    def n_win(self) -> int:
        return (self.np_per + P - 1) // P


FULL = Cfg()


def route_edges(edge_index: np.ndarray, cfg: Cfg = FULL):
    """Host-side routing (indices only). Returns (k_ovf, per_core):
    k_ovf[w] = overflow chunks for window w (max over cores, len n_win);
    per_core[p] = dict of index arrays for make_in_maps:
      id_col/id_slot/id_src  — stream position of each identity message
      ov_col/ov_slot/ov_src/ov_off/ov_dinv — same for overflow messages
      (cols are *local* chunk ids before k-table padding: filled in later)
      plus dinv (full-table) for the caller."""
    n = cfg.n_nodes
    nw = cfg.n_win
    src = np.asarray(edge_index[0], dtype=np.int64)
    dst = np.asarray(edge_index[1], dtype=np.int64)

    deg = (np.bincount(dst, minlength=n) + 1).astype(np.float32)
    dinv = (1.0 / np.sqrt(deg, dtype=np.float32)).astype(np.float32)

    # messages = self loops first (rank 0 within each dst), then edges
    loop = np.arange(n, dtype=np.int64)
    msrc = np.concatenate([loop, src])
    mdst = np.concatenate([loop, dst])
    order = np.argsort(mdst, kind="stable")
    s_dst = mdst[order]
    s_src = msrc[order]
    # rank of each message within its dst (loops got rank 0)
    starts = np.searchsorted(s_dst, np.arange(n))
    rank = np.arange(len(s_dst), dtype=np.int64) - starts[s_dst]

    per_core = []
    k_real = np.zeros((cfg.m, nw), np.int64)
    for p in range(cfg.m):
        base = p * cfg.np_per
        lo = np.searchsorted(s_dst, base)
        hi = np.searchsorted(s_dst, base + cfg.np_per)
        d_loc = s_dst[lo:hi] - base
        c_src = s_src[lo:hi]
        c_rank = rank[lo:hi]
        w = d_loc >> 7
        slot = d_loc & 127

        idm = c_rank < T_ID
        id_w = w[idm]
        id_chunk = c_rank[idm]  # chunk-in-window (0..T_ID-1)
        id_slot = slot[idm]
        id_src = c_src[idm]

        ovm = ~idm
        ov_w = w[ovm]  # sorted ascending (messages sorted by dst)
        ov_src = c_src[ovm]
        ov_dst = d_loc[ovm]
        wstart = np.searchsorted(ov_w, np.arange(nw))
        pos = np.arange(len(ov_w), dtype=np.int64) - wstart[ov_w]
        ov_chunk = pos >> 7
        ov_slot = pos & 127
        k_real[p] = np.ceil(np.bincount(ov_w, minlength=nw) / P).astype(np.int64)

        per_core.append(
            dict(
                id_w=id_w,
                id_chunk=id_chunk,
                id_slot=id_slot,
                id_src=id_src,
                ov_w=ov_w,
                ov_chunk=ov_chunk,
                ov_slot=ov_slot,
                ov_src=ov_src,
                ov_off=(ov_dst & 127),
                ov_dinv=dinv[ov_dst + base],
            )
        )

    k_ovf = k_real.max(axis=0)  # [n_win]
    # s_dst/s_src kept for the post-run sample check in kernel()
    per_core.append(dict(s_dst=s_dst, s_src=s_src))
    return k_ovf, per_core, dinv


def build_program(k_ovf, cfg: Cfg = FULL, sdt=mybir.dt.float16):
    """Build + compile the SPMD bass program (identical on all cores)."""
    nw = cfg.n_win
    k_ovf = np.asarray(k_ovf, dtype=np.int64)
    c_tot = int(nw * T_ID + k_ovf.sum())
    c_ovf = int(k_ovf.sum())
    n_grp = (nw + GRP - 1) // GRP

    nc = bacc.Bacc(
        "TRN2",
        target_bir_lowering=False,
        debug=False,
        enable_asserts=False,
        num_devices=cfg.m,
    )
    f32 = mybir.dt.float32
    stream_t = nc.dram_tensor("stream_t", [P, c_tot * P], sdt, kind="ExternalInput").ap()
    do_in = nc.dram_tensor("do_ovf", [P, max(c_ovf, 1)], f32, kind="ExternalInput").ap()
    nv_in = nc.dram_tensor("nv_ovf", [P, max(c_ovf, 1)], f32, kind="ExternalInput").ap()
    d2_in = nc.dram_tensor("d2", [P, nw], f32, kind="ExternalInput").ap()
    io_in = nc.dram_tensor("iota", [P, P], sdt, kind="ExternalInput").ap()
    ioc_in = nc.dram_tensor("iotac", [P, 1], f32, kind="ExternalInput").ap()
    w_in = nc.dram_tensor("w", [cfg.in_ch, cfg.out_ch], sdt, kind="ExternalInput").ap()
    b_in = nc.dram_tensor("b", [P, 1], f32, kind="ExternalInput").ap()
    out_t = nc.dram_tensor("out_t", [P, nw * P], sdt, kind="ExternalOutput").ap()

    with tile.TileContext(nc) as tc:
        with ExitStack() as ctx:
            cpool = ctx.enter_context(tc.tile_pool(name="const", bufs=1))
            gpool = ctx.enter_context(tc.tile_pool(name="gather", bufs=GBUFS))
            ohpool = ctx.enter_context(tc.tile_pool(name="oh", bufs=24))
            aggpool = ctx.enter_context(tc.tile_pool(name="agg", bufs=4))
            outpool = ctx.enter_context(tc.tile_pool(name="outp", bufs=4))
            pp1 = ctx.enter_context(tc.tile_pool(name="ps1", bufs=4, space="PSUM"))
            pp2 = ctx.enter_context(tc.tile_pool(name="ps2", bufs=2, space="PSUM"))

            do = cpool.tile([P, max(c_ovf, 1)], f32)
            nv = cpool.tile([P, max(c_ovf, 1)], f32)
            d2 = cpool.tile([P, nw], f32)
            io = cpool.tile([P, P], sdt)
            ioc = cpool.tile([P, 1], f32)
            wt = cpool.tile([P, cfg.out_ch], sdt)
            bb = cpool.tile([P, 1], f32)
            nc.sync.dma_start(out=do[:], in_=do_in[:])
            nc.sync.dma_start(out=nv[:], in_=nv_in[:])
            nc.sync.dma_start(out=d2[:], in_=d2_in[:])
            nc.sync.dma_start(out=io[:], in_=io_in[:])
            nc.sync.dma_start(out=ioc[:], in_=ioc_in[:])
            nc.sync.dma_start(out=wt[:], in_=w_in[:])
            nc.sync.dma_start(out=bb[:], in_=b_in[:])

            # one diag tile per window, built just-in-time (inline, so DVE
            # program order interleaves diag and overflow-oh builds); bufs=nw
            # so tiles are never recycled (no WAR serialization)
            dgpool = ctx.enter_context(tc.tile_pool(name="dg", bufs=nw))

            col = 0  # stream chunk column
            colk = 0  # overflow table column
            ndma = 0  # alternation counter for the two HWDGE engines
            # SDMA round-robins across ALL queued transfers at packet
            # granularity, so a big first slab behind a deep queue completes
            # late and stalls PE ~25us. Fetch the first SPLIT_G groups
            # per-window (small, early-completing slabs); group slabs after.
            SPLIT_G = 2
            for gi in range(n_grp):
                wls = list(range(gi * GRP, min((gi + 1) * GRP, nw)))
                gw = len(wls) * P
                # (tile, chunk base within tile) per window of this group
                tiles = []
                if gi < SPLIT_G:
                    for w in wls:
                        kw_tot = T_ID + int(k_ovf[w])
                        wt_t = gpool.tile([P, kw_tot * P], sdt)
                        deng = nc.sync if ndma % 2 == 0 else nc.scalar
                        ndma += 1
                        deng.dma_start(
                            out=wt_t[:],
                            in_=stream_t[:, col * P : (col + kw_tot) * P],
                        )
                        tiles.append((wt_t, 0))
                        col += kw_tot
                else:
                    kg = sum(T_ID + int(k_ovf[w]) for w in wls)
                    gt = gpool.tile([P, kg * P], sdt)
                    deng = nc.sync if ndma % 2 == 0 else nc.scalar
                    ndma += 1
                    deng.dma_start(
                        out=gt[:], in_=stream_t[:, col * P : (col + kg) * P]
                    )
                    base = 0
                    for w in wls:
                        tiles.append((gt, base))
                        base += T_ID + int(k_ovf[w])
                    col += kg
                ps1 = pp1.tile([P, gw], mybir.dt.float32, space="PSUM")
                for wl, w in enumerate(wls):
                    kw = int(k_ovf[w])
                    wtile, cc = tiles[wl]
                    dgt = dgpool.tile([P, P], sdt)
                    nc.vector.tensor_scalar(
                        out=dgt[:],
                        in0=io[:],
                        scalar1=ioc[:],
                        scalar2=d2[:, w : w + 1],
                        op0=mybir.AluOpType.is_equal,
                        op1=mybir.AluOpType.mult,
                    )
                    for k in range(T_ID):
                        nc.tensor.matmul(
                            ps1[:, wl * P : (wl + 1) * P],
                            lhsT=wtile[:, cc * P : (cc + 1) * P],
                            rhs=dgt[:],
                            start=(k == 0),
                            stop=(k == T_ID - 1 and kw == 0),
                        )
                        cc += 1
                    for c in range(kw):
                        oh = ohpool.tile([P, P], sdt)
                        nc.vector.tensor_scalar(
                            out=oh[:],
                            in0=io[:],
                            scalar1=do[:, colk + c : colk + c + 1],
                            scalar2=nv[:, colk + c : colk + c + 1],
                            op0=mybir.AluOpType.is_equal,
                            op1=mybir.AluOpType.mult,
                        )
                        nc.tensor.matmul(
                            ps1[:, wl * P : (wl + 1) * P],
                            lhsT=wtile[:, cc * P : (cc + 1) * P],
                            rhs=oh[:],
                            start=False,
                            stop=(c == kw - 1),
                        )
                        cc += 1
                    colk += kw
                agg = aggpool.tile([P, gw], sdt)
                nc.scalar.copy(out=agg[:], in_=ps1[:])
                ps2 = pp2.tile([P, gw], mybir.dt.float32, space="PSUM")
                nc.tensor.matmul(ps2[:], lhsT=wt[:], rhs=agg[:], start=True, stop=True)
                # batch output writes: 4 groups (16 windows, 512KB) per DMA —
                # 128KB out-DMAs only reach ~180GB/s on the SDMA engines
                if gi % OBATCH == 0:
                    ob_w0 = wls[0]
                    ob_wn = min(nw, ob_w0 + OBATCH * GRP)
                    obuf = outpool.tile([P, (ob_wn - ob_w0) * P], sdt)
                nc.scalar.activation(
                    out=obuf[:, (wls[0] - ob_w0) * P : (wls[0] - ob_w0) * P + gw],
                    in_=ps2[:],
                    func=mybir.ActivationFunctionType.Relu,
                    bias=bb[:],
                    scale=1.0,
                )
                if gi % OBATCH == OBATCH - 1 or gi == n_grp - 1:
                    oeng = (
                        nc.gpsimd
                        if OUT_ON_POOL
                        else (nc.scalar if gi % 2 == 0 else nc.sync)
                    )
                    oeng.dma_start(
                        out=out_t[:, ob_w0 * P : ob_wn * P], in_=obuf[:]
                    )

    nc.compile()
    return nc


def make_in_maps(x, W, b, k_ovf, per_core, dinv, cfg: Cfg = FULL, np_sdt=np.float16):
    nw = cfg.n_win
    k_ovf = np.asarray(k_ovf, dtype=np.int64)
    c_tot = int(nw * T_ID + k_ovf.sum())
    c_ovf = int(k_ovf.sum())
    # column base of window w's identity block in the stream; overflow block
    # follows immediately. Also the overflow-table column base per window.
    cumk = np.zeros(nw + 1, np.int64)
    np.cumsum(k_ovf, out=cumk[1:])
    col_base = T_ID * np.arange(nw, dtype=np.int64) + cumk[:-1]
    ovf_base = cumk[:-1]

    x32 = np.asarray(x, dtype=np.float32)
    x2 = (x32 * dinv[:, None]).astype(np_sdt)  # dinv[src]-prescaled table

    iota = np.broadcast_to(
        np.arange(P, dtype=np.float32), (P, P)
    ).astype(np_sdt).copy()
    iotac = np.arange(P, dtype=np.float32).reshape(P, 1).copy()
    w_np = np.ascontiguousarray(np.asarray(W, dtype=np.float32)).astype(np_sdt)
    b_np = np.asarray(b, dtype=np.float32).reshape(P, 1).copy()

    in_maps = []
    for p in range(cfg.m):
        r = per_core[p]
        base = p * cfg.np_per
        stream = np.zeros((c_tot, P, cfg.in_ch), np_sdt)
        icol = col_base[r["id_w"]] + r["id_chunk"]
        stream[icol, r["id_slot"]] = x2[r["id_src"]]
        ocol = col_base[r["ov_w"]] + T_ID + r["ov_chunk"]
        stream[ocol, r["ov_slot"]] = x2[r["ov_src"]]
        stream_t = np.ascontiguousarray(
            stream.transpose(1, 0, 2).reshape(P, c_tot * cfg.in_ch)
        )

        do_np = np.zeros((P, max(c_ovf, 1)), np.float32)
        nv_np = np.zeros((P, max(c_ovf, 1)), np.float32)
        okol = ovf_base[r["ov_w"]] + r["ov_chunk"]
        do_np[r["ov_slot"], okol] = r["ov_off"].astype(np.float32)
        nv_np[r["ov_slot"], okol] = r["ov_dinv"]

        d2_np = np.zeros((P, nw), np.float32)
        nn = cfg.np_per
        loc = np.arange(nn, dtype=np.int64)
        d2_np[loc & 127, loc >> 7] = dinv[base + loc]

        in_maps.append(
            dict(
                stream_t=stream_t,
                do_ovf=do_np,
                nv_ovf=nv_np,
                d2=d2_np,
                iota=iota,
                iotac=iotac,
                w=w_np,
                b=b_np,
            )
        )
    return in_maps


_PROG_CACHE = {}


def _sample_check(out, x, W, b, dinv, s_dst, s_src, n_samples=512, seed=7):
    """Host-recompute a random sample of output rows; returns True if the
    device output matches (guards against rare first-run DMA/engine races)."""
    n = out.shape[0]
    rng = np.random.default_rng(seed)
    samp = rng.choice(n, size=n_samples, replace=False)
    x32 = np.asarray(x, dtype=np.float32)
    w32 = np.asarray(W, dtype=np.float32)
    b32 = np.asarray(b, dtype=np.float32)
    starts = np.searchsorted(s_dst, samp)
    ends = np.searchsorted(s_dst, samp + 1)
    for d, lo, hi in zip(samp, starts, ends):
        srcs = s_src[lo:hi]
        agg = (x32[srcs] * dinv[srcs][:, None]).sum(axis=0) * dinv[d]
        exp = np.maximum(agg @ w32 + b32, 0.0)
        scale = max(float(np.linalg.norm(exp)), 1e-3)
        if float(np.linalg.norm(out[d] - exp)) > 0.02 * scale:
            return False
    return True


def kernel(x, edge_index, W, b):
    cfg = FULL
    k_ovf, per_core, dinv = route_edges(edge_index, cfg)
    aux = per_core[cfg.m]  # s_dst/s_src appended by route_edges
    key = (tuple(int(v) for v in k_ovf), cfg)
    if key not in _PROG_CACHE:
        _PROG_CACHE[key] = build_program(k_ovf, cfg)
    nc = _PROG_CACHE[key]
    in_maps = make_in_maps(x, W, b, k_ovf, per_core, dinv, cfg)
    out = np.empty((cfg.n_nodes, cfg.out_ch), np.float32)
    for attempt in range(3):
        res = run_bass_kernel_spmd(nc, in_maps, core_ids=list(range(cfg.m)))
        for p in range(cfg.m):
            out[p * cfg.np_per : (p + 1) * cfg.np_per] = (
                res.results[p]["out_t"][:, : cfg.np_per].T.astype(np.float32)
            )
        if _sample_check(out, x, W, b, dinv, aux["s_dst"], aux["s_src"]):
            break
        print(f"kernel: sample check failed (attempt {attempt}), re-running", flush=True)
    return out


# revision 36
# speedup vs baseline: 1.2894x; 1.2513x over previous
r"""GCN block (gather -> normalize -> scatter-add -> linear -> relu) on 8 trn2 cores.

Math: out = relu( \hat{A} (X W) + b ) with \hat{A} = D^-1/2 (A + I) D^-1/2,
degree over destination of (edges + self loops).

v12 "materialized identity-stream, dinv-folded, split fp16/fp8" design:
  norm(e) = dinv[src]*dinv[dst] is folded ENTIRELY into the host-built
  message rows (each stream row is one message, so both factors are
  per-row scalars): row = x[src]*dinv[src]*dinv[dst]. The scatter matmul
  rhs is then a CONSTANT exact 0/1 identity — representable in fp8 — and
  the stream can be split by message rank into an fp16 stream (first
  S_FP16 chunks per window, incl. the self loop) and an fp8-e4m3 stream
  (remaining identity chunks + overflow), halving those bytes. fp8
  quantization noise is incoherent per-row (~1.8%*sqrt(f) overall, f =
  fp8 fraction of message energy); measured end-to-end rel err 1.45e-2 (S=10)
  budget-checked against the 2e-2 gate via the numpy emulator before
  shipping. Self loops ride chunk 0 (fp16).

  Host routing (per core, 12500 dst nodes = 98 windows of 128):
   - message m = k-th in-message of dst d (self loop first): k < S_FP16
     -> fp16 identity chunk k at slot d%128; S_FP16 <= k < T_ID -> fp8
     identity chunk; k >= T_ID -> per-window dense fp8 OVERFLOW chunks
     with a 0/1 one-hot rhs (iota==dst_off) built by DVE tensor_scalar.
   - both streams are materialized transposed ([128 slots, C*128 ch]) so
     the device "gather" is plain sequential HWDGE dma_starts at full HBM
     bandwidth (~375GB/s measured) — no SWDGE descriptor-issue bottleneck
     (which limited the per-edge indirect-DMA design to ~1.25ms).

  Device per PSUM group (4 windows = one 512-col PSUM bank):
   - 2 dma_starts (fp16 + fp8 slab) alternating the two HWDGE engines
     SP/ACT; first SPLIT_G groups fetch per-window (SDMA round-robins
     across ALL queued transfers, so small early slabs complete fast and
     cut the PE startup stall)
   - per window: S_FP16 fp16 matmuls (rhs = const identity fp16), then
     fp8 identity matmuls (rhs = const identity fp8), then overflow fp8
     matmuls (rhs = one-hot, DVE-built; NOT gpsimd — Pool tensor_scalar
     is a ~2.2us software op), all accumulating ps1[ch, dst] in fp32
   - epilogue: ps1 -> fp16 agg (ACT copy), ps2 = W^T-form matmul,
     relu+bias on ACT, out DMA [ch, dst] fp16; host transposes/casts.

Program shape depends only on the cross-core per-window overflow chunk
counts (k-table); S_FP16/T_ID are fixed.

kernel() re-verifies each device run against a 512-row host recompute
(loose 15%-per-row gate: fp8 noise is legit, corruption is O(50%)) and
retries — a rare first-execution DMA race produced one corrupted run
during development.
"""

import sys
from contextlib import ExitStack
from dataclasses import dataclass

import numpy as np

if "/opt/trn_rl_repo" not in sys.path:
    sys.path.insert(0, "/opt/trn_rl_repo")

import concourse.bass as bass
import concourse.bacc as bacc
import concourse.mybir as mybir
import concourse.tile as tile
from concourse.bass_utils import run_bass_kernel_spmd


def _ensure_axon_hooks_stub():
    """The image's antenv package lacks axon_hooks; bass_utils imports it on
    the trace path (e.g. when BASS_TRACE is set). Provide a stub returning
    None so tracing degrades gracefully instead of raising ImportError."""
    import types

    name = "antenv.axon_hooks"
    if name in sys.modules:
        return
    try:
        __import__(name)
        return
    except ImportError:
        pass
    mod = types.ModuleType(name)
    mod._hook = None
    mod.set_axon_ntff_profile_hook = lambda h: setattr(mod, "_hook", h)
    mod.get_axon_ntff_profile_hook = lambda: mod._hook
    sys.modules[name] = mod
    try:
        import antenv

        antenv.axon_hooks = mod
    except ImportError:
        pass


_ensure_axon_hooks_stub()

P = 128
T_ID = 14  # identity chunks per window (first T_ID msgs of each dst)
S_FP16 = 10  # of which the first S_FP16 are fp16; rest + overflow are fp8
GRP = 4  # windows per PSUM group
SPLIT_G = 2  # leading groups fetched per-window for a fast pipeline start
GBUFS = 6  # stream slab pool depth (per stream)
# fp8 rows are pre-scaled by Q8_SCALE on the host (row values ~0.05 would
# otherwise land in e4m3's subnormal range and lose mantissa bits); the
# exact power-of-two 1/Q8_SCALE is folded into the fp8 identity/one-hot rhs
Q8_SCALE = 32.0


@dataclass(frozen=True)
class Cfg:
    n_nodes: int = 100000
    in_ch: int = 128
    out_ch: int = 128
    m: int = 8  # cores

    @property
    def np_per(self) -> int:
        return self.n_nodes // self.m

    @property
    def n_win(self) -> int:
        return (self.np_per + P - 1) // P


FULL = Cfg()


def route_edges(edge_index: np.ndarray, cfg: Cfg = FULL):
    """Host-side routing (indices only). Returns (k_ovf, per_core, dinv):
    k_ovf[w] = overflow chunks for window w (max over cores, len n_win);
    per_core[p] = index arrays for make_in_maps; per_core[m] = the sorted
    (s_dst, s_src) message lists for kernel()'s sample check."""
    n = cfg.n_nodes
    nw = cfg.n_win
    src = np.asarray(edge_index[0], dtype=np.int64)
    dst = np.asarray(edge_index[1], dtype=np.int64)

    deg = (np.bincount(dst, minlength=n) + 1).astype(np.float32)
    dinv = (1.0 / np.sqrt(deg, dtype=np.float32)).astype(np.float32)

    # messages = edges + self loops; within each dst, rank messages by
    # DESCENDING dinv[src] so the low-energy tail lands in the fp8 chunks
    # (rank >= S_FP16): fp8 noise is proportional to quantized row energy,
    # and rank assignment is free (any order sums the same).
    loop = np.arange(n, dtype=np.int64)
    msrc = np.concatenate([loop, src])
    mdst = np.concatenate([loop, dst])
    order = np.lexsort((-dinv[msrc], mdst))
    s_dst = mdst[order]
    s_src = msrc[order]
    starts = np.searchsorted(s_dst, np.arange(n))
    rank = np.arange(len(s_dst), dtype=np.int64) - starts[s_dst]

    per_core = []
    k_real = np.zeros((cfg.m, nw), np.int64)
    for p in range(cfg.m):
        base = p * cfg.np_per
        lo = np.searchsorted(s_dst, base)
        hi = np.searchsorted(s_dst, base + cfg.np_per)
        d_loc = s_dst[lo:hi] - base
        c_src = s_src[lo:hi]
        c_rank = rank[lo:hi]
        w = d_loc >> 7
        slot = d_loc & 127

        idm = c_rank < T_ID
        ovm = ~idm
        ov_w = w[ovm]  # sorted ascending (messages sorted by dst)
        ov_dst = d_loc[ovm]
        wstart = np.searchsorted(ov_w, np.arange(nw))
        pos = np.arange(len(ov_w), dtype=np.int64) - wstart[ov_w]
        k_real[p] = np.ceil(np.bincount(ov_w, minlength=nw) / P).astype(np.int64)

        per_core.append(
            dict(
                id_w=w[idm],
                id_chunk=c_rank[idm],
                id_slot=slot[idm],
                id_src=c_src[idm],
                id_dst=d_loc[idm] + base,
                ov_w=ov_w,
                ov_chunk=pos >> 7,
                ov_slot=pos & 127,
                ov_src=c_src[ovm],
                ov_off=(ov_dst & 127),
                ov_dst=ov_dst + base,
            )
        )

    k_ovf = k_real.max(axis=0)  # [n_win]
    per_core.append(dict(s_dst=s_dst, s_src=s_src))
    return k_ovf, per_core, dinv


def build_program(k_ovf, cfg: Cfg = FULL, sdt=mybir.dt.float16, qdt=mybir.dt.float8e4):
    """Build + compile the SPMD bass program (identical on all cores)."""
    nw = cfg.n_win
    k_ovf = np.asarray(k_ovf, dtype=np.int64)
    c16 = nw * S_FP16
    c8 = int(nw * (T_ID - S_FP16) + k_ovf.sum())
    c_ovf = int(k_ovf.sum())
    n_grp = (nw + GRP - 1) // GRP

    nc = bacc.Bacc(
        "TRN2",
        target_bir_lowering=False,
        debug=False,
        enable_asserts=False,
        num_devices=cfg.m,
    )
    f32 = mybir.dt.float32
    st16 = nc.dram_tensor("stream16_t", [P, c16 * P], sdt, kind="ExternalInput").ap()
    st8 = nc.dram_tensor("stream8_t", [P, c8 * P], qdt, kind="ExternalInput").ap()
    do_in = nc.dram_tensor("do_ovf", [P, max(c_ovf, 1)], f32, kind="ExternalInput").ap()
    nv_in = nc.dram_tensor("nv_ovf", [P, max(c_ovf, 1)], f32, kind="ExternalInput").ap()
    io_in = nc.dram_tensor("iota", [P, P], sdt, kind="ExternalInput").ap()
    id16_in = nc.dram_tensor("ident16", [P, P], sdt, kind="ExternalInput").ap()
    id8_in = nc.dram_tensor("ident8", [P, P], qdt, kind="ExternalInput").ap()
    w_in = nc.dram_tensor("w", [cfg.in_ch, cfg.out_ch], sdt, kind="ExternalInput").ap()
    b_in = nc.dram_tensor("b", [P, 1], f32, kind="ExternalInput").ap()
    out_t = nc.dram_tensor("out_t", [P, nw * P], sdt, kind="ExternalOutput").ap()

    with tile.TileContext(nc) as tc:
        with ExitStack() as ctx:
            cpool = ctx.enter_context(tc.tile_pool(name="const", bufs=1))
            g16pool = ctx.enter_context(tc.tile_pool(name="g16", bufs=GBUFS))
            g8pool = ctx.enter_context(tc.tile_pool(name="g8", bufs=GBUFS))
            ohpool = ctx.enter_context(tc.tile_pool(name="oh", bufs=24))
            aggpool = ctx.enter_context(tc.tile_pool(name="agg", bufs=4))
            outpool = ctx.enter_context(tc.tile_pool(name="outp", bufs=4))
            pp1 = ctx.enter_context(tc.tile_pool(name="ps1", bufs=4, space="PSUM"))
            pp2 = ctx.enter_context(tc.tile_pool(name="ps2", bufs=2, space="PSUM"))

            do = cpool.tile([P, max(c_ovf, 1)], f32)
            nv = cpool.tile([P, max(c_ovf, 1)], f32)
            io = cpool.tile([P, P], sdt)
            id16c = cpool.tile([P, P], sdt)
            id8c = cpool.tile([P, P], qdt)
            wt = cpool.tile([P, cfg.out_ch], sdt)
            bb = cpool.tile([P, 1], f32)
            nc.sync.dma_start(out=do[:], in_=do_in[:])
            nc.sync.dma_start(out=nv[:], in_=nv_in[:])
            nc.sync.dma_start(out=io[:], in_=io_in[:])
            nc.sync.dma_start(out=id16c[:], in_=id16_in[:])
            nc.sync.dma_start(out=id8c[:], in_=id8_in[:])
            nc.sync.dma_start(out=wt[:], in_=w_in[:])
            nc.sync.dma_start(out=bb[:], in_=b_in[:])

            col16 = 0  # fp16 stream chunk column
            col8 = 0  # fp8 stream chunk column
            colk = 0  # overflow table column
            ndma = 0
            n8 = T_ID - S_FP16
            for gi in range(n_grp):
                wls = list(range(gi * GRP, min((gi + 1) * GRP, nw)))
                gw = len(wls) * P
                # (tile16, base16, tile8, base8) per window
                tiles = []
                if gi < SPLIT_G:
                    for w in wls:
                        k8 = n8 + int(k_ovf[w])
                        t16 = g16pool.tile([P, S_FP16 * P], sdt)
                        (nc.sync if ndma % 2 == 0 else nc.scalar).dma_start(
                            out=t16[:],
                            in_=st16[:, col16 * P : (col16 + S_FP16) * P],
                        )
                        ndma += 1
                        t8 = g8pool.tile([P, k8 * P], qdt)
                        (nc.sync if ndma % 2 == 0 else nc.scalar).dma_start(
                            out=t8[:], in_=st8[:, col8 * P : (col8 + k8) * P]
                        )
                        ndma += 1
                        tiles.append((t16, 0, t8, 0))
                        col16 += S_FP16
                        col8 += k8
                else:
                    kg16 = len(wls) * S_FP16
                    kg8 = sum(n8 + int(k_ovf[w]) for w in wls)
                    gt16 = g16pool.tile([P, kg16 * P], sdt)
                    (nc.sync if ndma % 2 == 0 else nc.scalar).dma_start(
                        out=gt16[:], in_=st16[:, col16 * P : (col16 + kg16) * P]
                    )
                    ndma += 1
                    gt8 = g8pool.tile([P, kg8 * P], qdt)
                    (nc.sync if ndma % 2 == 0 else nc.scalar).dma_start(
                        out=gt8[:], in_=st8[:, col8 * P : (col8 + kg8) * P]
                    )
                    ndma += 1
                    b16 = b8 = 0
                    for w in wls:
                        tiles.append((gt16, b16, gt8, b8))
                        b16 += S_FP16
                        b8 += n8 + int(k_ovf[w])
                    col16 += kg16
                    col8 += kg8
                ps1 = pp1.tile([P, gw], mybir.dt.float32, space="PSUM")
                for wl, w in enumerate(wls):
                    kw = int(k_ovf[w])
                    t16, b16, t8, b8 = tiles[wl]
                    reg = ps1[:, wl * P : (wl + 1) * P]
                    for k in range(S_FP16):
                        nc.tensor.matmul(
                            reg,
                            lhsT=t16[:, (b16 + k) * P : (b16 + k + 1) * P],
                            rhs=id16c[:],
                            start=(k == 0),
                            stop=False,
                        )
                    for k in range(n8):
                        nc.tensor.matmul(
                            reg,
                            lhsT=t8[:, (b8 + k) * P : (b8 + k + 1) * P],
                            rhs=id8c[:],
                            start=False,
                            stop=(k == n8 - 1 and kw == 0),
                        )
                    for c in range(kw):
                        oh = ohpool.tile([P, P], qdt)
                        nc.vector.tensor_scalar(
                            out=oh[:],
                            in0=io[:],
                            scalar1=do[:, colk + c : colk + c + 1],
                            scalar2=nv[:, colk + c : colk + c + 1],
                            op0=mybir.AluOpType.is_equal,
                            op1=mybir.AluOpType.mult,
                        )
                        nc.tensor.matmul(
                            reg,
                            lhsT=t8[:, (b8 + n8 + c) * P : (b8 + n8 + c + 1) * P],
                            rhs=oh[:],
                            start=False,
                            stop=(c == kw - 1),
                        )
                    colk += kw
                agg = aggpool.tile([P, gw], sdt)
                nc.scalar.copy(out=agg[:], in_=ps1[:])
                ps2 = pp2.tile([P, gw], mybir.dt.float32, space="PSUM")
                nc.tensor.matmul(ps2[:], lhsT=wt[:], rhs=agg[:], start=True, stop=True)
                ot = outpool.tile([P, gw], sdt)
                nc.scalar.activation(
                    out=ot[:],
                    in_=ps2[:],
                    func=mybir.ActivationFunctionType.Relu,
                    bias=bb[:],
                    scale=1.0,
                )
                (nc.scalar if gi % 2 == 0 else nc.sync).dma_start(
                    out=out_t[:, wls[0] * P : (wls[0] + len(wls)) * P], in_=ot[:]
                )

    nc.compile()
    return nc


def make_in_maps(
    x, W, b, k_ovf, per_core, dinv, cfg: Cfg = FULL,
    np_sdt=np.float16, np_qdt=mybir.dt.np(mybir.dt.float8e4),
):
    nw = cfg.n_win
    k_ovf = np.asarray(k_ovf, dtype=np.int64)
    n8 = T_ID - S_FP16
    c16 = nw * S_FP16
    c8 = int(nw * n8 + k_ovf.sum())
    c_ovf = int(k_ovf.sum())
    cumk = np.zeros(nw + 1, np.int64)
    np.cumsum(k_ovf, out=cumk[1:])
    cb8 = n8 * np.arange(nw, dtype=np.int64) + cumk[:-1]  # fp8 col base per win
    ovf_base = cumk[:-1]

    x2 = np.asarray(x, dtype=np.float32) * dinv[:, None]  # dinv[src] folded

    iota = np.broadcast_to(np.arange(P, dtype=np.float32), (P, P)).astype(np_sdt).copy()
    ident = np.eye(P, dtype=np.float32)
    w_np = np.ascontiguousarray(np.asarray(W, dtype=np.float32)).astype(np_sdt)
    b_np = np.asarray(b, dtype=np.float32).reshape(P, 1).copy()

    in_maps = []
    for p in range(cfg.m):
        r = per_core[p]
        # full norm folded into the rows: x * dinv[src] * dinv[dst]
        id_rows = x2[r["id_src"]] * dinv[r["id_dst"]][:, None]
        ov_rows = (x2[r["ov_src"]] * dinv[r["ov_dst"]][:, None]) * Q8_SCALE

        i16 = r["id_chunk"] < S_FP16
        stream16 = np.zeros((c16, P, cfg.in_ch), np_sdt)
        stream16[
            S_FP16 * r["id_w"][i16] + r["id_chunk"][i16], r["id_slot"][i16]
        ] = id_rows[i16].astype(np_sdt)

        i8 = ~i16
        stream8 = np.zeros((c8, P, cfg.in_ch), np_qdt)
        stream8[
            cb8[r["id_w"][i8]] + (r["id_chunk"][i8] - S_FP16), r["id_slot"][i8]
        ] = (id_rows[i8] * Q8_SCALE).astype(np_qdt)
        stream8[
            cb8[r["ov_w"]] + n8 + r["ov_chunk"], r["ov_slot"]
        ] = ov_rows.astype(np_qdt)

        st16_t = np.ascontiguousarray(
            stream16.transpose(1, 0, 2).reshape(P, c16 * cfg.in_ch)
        )
        st8_t = np.ascontiguousarray(
            stream8.transpose(1, 0, 2).reshape(P, c8 * cfg.in_ch)
        )

        do_np = np.zeros((P, max(c_ovf, 1)), np.float32)
        nv_np = np.zeros((P, max(c_ovf, 1)), np.float32)
        okol = ovf_base[r["ov_w"]] + r["ov_chunk"]
        do_np[r["ov_slot"], okol] = r["ov_off"].astype(np.float32)
        nv_np[r["ov_slot"], okol] = 1.0 / Q8_SCALE

        in_maps.append(
            dict(
                stream16_t=st16_t,
                stream8_t=st8_t,
                do_ovf=do_np,
                nv_ovf=nv_np,
                iota=iota,
                ident16=ident.astype(np_sdt),
                ident8=(ident / Q8_SCALE).astype(np_qdt),
                w=w_np,
                b=b_np,
            )
        )
    return in_maps


_PROG_CACHE = {}


def _sample_check(out, x, W, b, dinv, s_dst, s_src, n_samples=512, seed=7):
    """Host-recompute a random sample of output rows; returns True if the
    device output matches within the fp8-noise budget (guards against rare
    first-run DMA/engine races, which corrupt rows at O(50%) level)."""
    n = out.shape[0]
    rng = np.random.default_rng(seed)
    samp = rng.choice(n, size=n_samples, replace=False)
    x32 = np.asarray(x, dtype=np.float32)
    w32 = np.asarray(W, dtype=np.float32)
    b32 = np.asarray(b, dtype=np.float32)
    starts = np.searchsorted(s_dst, samp)
    ends = np.searchsorted(s_dst, samp + 1)
    for d, lo, hi in zip(samp, starts, ends):
        srcs = s_src[lo:hi]
        agg = (x32[srcs] * dinv[srcs][:, None]).sum(axis=0) * dinv[d]
        exp = np.maximum(agg @ w32 + b32, 0.0)
        scale = max(float(np.linalg.norm(exp)), 1e-3)
        if float(np.linalg.norm(out[d] - exp)) > 0.15 * scale:
            return False
    return True


def kernel(x, edge_index, W, b):
    cfg = FULL
    k_ovf, per_core, dinv = route_edges(edge_index, cfg)
    aux = per_core[cfg.m]  # s_dst/s_src appended by route_edges
    key = (tuple(int(v) for v in k_ovf), cfg)
    if key not in _PROG_CACHE:
        _PROG_CACHE[key] = build_program(k_ovf, cfg)
    nc = _PROG_CACHE[key]
    in_maps = make_in_maps(x, W, b, k_ovf, per_core, dinv, cfg)
    out = np.empty((cfg.n_nodes, cfg.out_ch), np.float32)
    for attempt in range(3):
        res = run_bass_kernel_spmd(nc, in_maps, core_ids=list(range(cfg.m)))
        for p in range(cfg.m):
            out[p * cfg.np_per : (p + 1) * cfg.np_per] = (
                res.results[p]["out_t"][:, : cfg.np_per].T.astype(np.float32)
            )
        if _sample_check(out, x, W, b, dinv, aux["s_dst"], aux["s_src"]):
            break
        print(f"kernel: sample check failed (attempt {attempt}), re-running", flush=True)
    return out
